# revision 1
# baseline (speedup 1.0000x reference)
"""Causal self-attention Trainium2 kernel (8 NeuronCores).

Sharding: tensor-parallel over heads x data-parallel over batch.
Core c handles batch b = c // 4 and head group g = c % 4 (4 heads of 16).
Each core computes q/k/v projections for its heads, causal attention, and a
partial output projection (its 256 columns of the 1024-wide contraction);
the host sums the 4 partials per batch.

Layout strategy (all transpose-free on device):
  - q,k are projected directly in transposed layout qkT[e, t] (e on
    partitions) so they feed the scores matmul as lhsT/rhs.
  - scores are computed transposed, sT[k_chunk=128, q_block=512], one
    matmul per (k_chunk, q_block) with K=hd=64.
  - softmax: no max-subtraction (scores ~ N(0,1), exp is safe in fp32);
    exp on ScalarE reading PSUM; causal mask added as -1e9 bias into PSUM
    for diagonal chunks; denominator comes free as an extra ones-column in
    the PV matmul's lhsT.
  - v is projected in natural layout v[t, hd] which is exactly the PV lhsT.
  - PV output yT[hd, q] is normalized via a K=1 broadcast matmul of the
    reciprocal row, then used directly as the proj lhsT.
All matmuls run as float32r (full PE rate at N>=256).
"""

import numpy as np

import concourse.bass as bass
from concourse import bacc
import concourse.mybir as mybir
import concourse.tile as tile
from concourse.bass_utils import run_bass_kernel_spmd

B, T, D, H = 2, 2048, 1024, 16
HD = D // H          # 64
HPC = 4              # heads per core
NCORES = 8
EQK = 2 * HPC * HD   # 512 rows of q+k per core
EV = HPC * HD        # 256 rows of v per core
TB = 512             # t/q block
NTB = T // TB        # 4
TC = 128             # t chunk
NTC = T // TC        # 16
DCH = D // 128       # 8 contraction chunks
F32 = mybir.dt.float32
F32R = mybir.dt.float32r

_cache = {}


def _ensure_ntff_hook():
    """The agent image's antenv lacks axon_hooks; fabricate it so
    run_bass_kernel_spmd(trace=True) can capture NTFF profiles."""
    import sys
    import types
    try:
        import antenv.axon_hooks  # noqa: F401
        return
    except ImportError:
        pass
    try:
        import antenv
        from trn_agent_boot.trn_boot import _ntff_profile_via_ctypes
        hook = {"h": _ntff_profile_via_ctypes("/opt/axon/libaxon_pjrt.so")}
        m = types.ModuleType("antenv.axon_hooks")
        m.get_axon_ntff_profile_hook = lambda: hook["h"]
        m.set_axon_ntff_profile_hook = lambda h: hook.update(h=h)
        sys.modules["antenv.axon_hooks"] = m
        antenv.axon_hooks = m
    except Exception:
        pass


def _build_nc():
    nc = bacc.Bacc("TRN2", target_bir_lowering=False, debug=False,
                  num_devices=NCORES)
    xT = nc.dram_tensor("xT", [D, T], F32R, kind="ExternalInput")
    wqk = nc.dram_tensor("wqk", [D, EQK], F32R, kind="ExternalInput")
    wv = nc.dram_tensor("wv", [D, EV], F32R, kind="ExternalInput")
    wp = nc.dram_tensor("wp", [EV, D], F32R, kind="ExternalInput")
    masks = nc.dram_tensor("masks", [4, 128, TB], F32, kind="ExternalInput")
    onesd = nc.dram_tensor("onesd", [128, HD], F32R, kind="ExternalInput")
    out = nc.dram_tensor("out", [T, D], F32, kind="ExternalOutput")

    with tile.TileContext(nc) as tc:
        with (
            nc.allow_low_precision(reason="fp32r matmul inputs; psum stays fp32"),
            tc.tile_pool(name="persist", bufs=1) as persist,
            tc.tile_pool(name="xin", bufs=2) as xin,
            tc.tile_pool(name="work", bufs=3) as work,
            tc.tile_pool(name="probsp", bufs=8) as probsp,
            tc.tile_pool(name="outp", bufs=3) as outp,
            tc.tile_pool(name="ps_big", bufs=4, space="PSUM") as ps_big,
            tc.tile_pool(name="ps_acc", bufs=4, space="PSUM") as ps_acc,
        ):
            # ---- persistent SBUF tensors ----
            wqk_sb = persist.tile([128, DCH, EQK], F32R)   # 16KB/part
            nc.sync.dma_start(wqk_sb[:], wqk.rearrange("(c p) e -> p c e", p=128))
            wv_sb = persist.tile([128, DCH, EV], F32R)     # 8KB/part
            nc.sync.dma_start(wv_sb[:], wv.rearrange("(c p) e -> p c e", p=128))
            wp_sb = persist.tile([128, 2, D], F32R)        # 8KB/part
            nc.sync.dma_start(wp_sb[:], wp.rearrange("(c p) e -> p c e", p=128))
            mask_sb = persist.tile([128, 4, TB], F32)     # 8KB/part
            nc.sync.dma_start(mask_sb[:], masks.rearrange("j p q -> p j q"))

            # qkT[e, t]: 4 chunks of 128 e-rows (q heads 01, q heads 23,
            # k heads 01, k heads 23), each [128, T]
            qkT = [persist.tile([128, T], F32R, tag=f"qkT{i}", name=f"qkT{i}")
                   for i in range(4)]
            # v_sb[t_chunk]: [128, h, 65]; col 64 of each head slot is 1.0
            v_sb = [persist.tile([128, HPC, HD + 1], F32R, tag=f"v{i}",
                                name=f"v{i}")
                    for i in range(NTC)]
            # yT: unnormalized-then-normalized attention output, [hd_all, t]
            yT = [persist.tile([128, T], F32R, tag=f"yT{i}", name=f"yT{i}")
                  for i in range(2)]

            def qT_ap(h):  # [64, T]
                return qkT[h // 2][64 * (h % 2):64 * (h % 2) + 64, :]

            def kT_ap(h):  # [64, T]
                return qkT[2 + h // 2][64 * (h % 2):64 * (h % 2) + 64, :]

            # ================= QKV projection =================
            for b in range(NTB):
                x_t = xin.tile([128, DCH, TB], F32R, tag="x")
                nc.sync.dma_start(
                    x_t[:], xT[:, b * TB:(b + 1) * TB]
                    .rearrange("(c p) t -> p c t", p=128))
                # q,k in transposed layout: psum[e_chunk 128, t 512]
                for ec in range(4):
                    ps = ps_big.tile([128, TB], F32, tag="mm", name="ps_qk")
                    for dc in range(DCH):
                        nc.tensor.matmul(
                            ps[:],
                            (wqk_sb[:, dc, 128 * ec:128 * (ec + 1)]),
                            (x_t[:, dc, :]),
                            start=(dc == 0), stop=(dc == DCH - 1))
                    nc.scalar.copy(qkT[ec][:, b * TB:(b + 1) * TB], ps[:])
                # v in natural layout: psum[t_chunk 128, hd 256]
                for t2 in range(4):
                    tc_i = 4 * b + t2
                    ps = ps_big.tile([128, TB], F32, tag="mm", name="ps_v")
                    for dc in range(DCH):
                        nc.tensor.matmul(
                            ps[:, 0:EV],
                            (x_t[:, dc, 128 * t2:128 * (t2 + 1)]),
                            (wv_sb[:, dc, :]),
                            start=(dc == 0), stop=(dc == DCH - 1))
                    nc.vector.tensor_copy(
                        v_sb[tc_i][:, :, 0:HD],
                        ps[:, 0:EV].rearrange("p (h f) -> p h f", h=HPC))
                    nc.sync.dma_start(v_sb[tc_i][:, :, HD], onesd[:, 0:HPC])

            # ================= attention =================
            # kc-outer / h-inner: PE sees 4 independent chains per round,
            # so each PV matmul's exp() has ~3 matmuls of latency cover.
            for b in range(NTB):
                nk = 4 * b + 4
                ps_pvs = [ps_acc.tile([HD + 1, TB], F32, tag="pv",
                                      name=f"pv_{b}_{h}") for h in range(HPC)]
                for kc in range(nk):
                    diag = kc >= 4 * b
                    # 4 sT matmuls back-to-back: head pairs live at base
                    # partitions 0/64 of their qkT chunk, so adjacent mms
                    # run concurrently in opposite PE row-groups.
                    ps_ss = []
                    for h in range(HPC):
                        ps_s = ps_big.tile([128, TB], F32, tag="mm",
                                           name=f"ps_s{h}")
                        nc.tensor.matmul(
                            ps_s[:],
                            (kT_ap(h)[:, 128 * kc:128 * (kc + 1)]),
                            (qT_ap(h)[:, b * TB:(b + 1) * TB]),
                            start=True, stop=True)
                        ps_ss.append(ps_s)
                    probss = []
                    for h in range(HPC):
                        probs = probsp.tile([128, TB], F32R, tag="probs",
                                            name=f"probs{h}")
                        nc.scalar.activation(
                            probs[:], ps_ss[h][:],
                            mybir.ActivationFunctionType.Exp,
                            scale=1.0 / np.sqrt(HD))
                        if diag:
                            nc.vector.tensor_mul(
                                probs[:], probs[:],
                                mask_sb[:, kc - 4 * b, :])
                        probss.append(probs)
                    for h in range(HPC):
                        nc.tensor.matmul(
                            ps_pvs[h][:],
                            (v_sb[kc][:, h, :]),
                            (probss[h][:]),
                            start=(kc == 0), stop=(kc == nk - 1))
                # tail: drain PSUM fast (frees pv slots), then normalize
                # yT in SBUF off the critical path.
                dens = []
                for h in range(HPC):
                    yslice = yT[h // 2][64 * (h % 2):64 * (h % 2) + 64,
                                        b * TB:(b + 1) * TB]
                    nc.vector.tensor_copy(yslice, ps_pvs[h][0:HD, :])
                    den = work.tile([1, TB], F32, tag="den", name=f"den{h}")
                    nc.scalar.copy(den[:], ps_pvs[h][HD:HD + 1, :])
                    dens.append(den)
                for h in range(HPC):
                    rec = work.tile([1, TB], F32, tag="rec", name=f"rec{h}")
                    nc.vector.reciprocal_approx_fast(rec[:], dens[h][:])
                    bc_sb = work.tile([128, TB], F32, tag="bc_sb")
                    nc.gpsimd.partition_broadcast(bc_sb[:], rec[:])
                    off = 64 * (h % 2)
                    yslice = yT[h // 2][off:off + 64,
                                        b * TB:(b + 1) * TB]
                    nc.vector.tensor_mul(yslice, yslice,
                                         bc_sb[off:off + 64, :])

            # ================= output projection =================
            for tc_i in range(NTC):
                for e in range(2):
                    ps = ps_big.tile([128, TB], F32, tag="mm", name="ps_proj")
                    for c in range(2):
                        nc.tensor.matmul(
                            ps[:],
                            (yT[c][:, 128 * tc_i:128 * (tc_i + 1)]),
                            (wp_sb[:, c, 512 * e:512 * (e + 1)]),
                            start=(c == 0), stop=(c == 1))
                    o_sb = outp.tile([128, TB], F32, tag="o")
                    nc.vector.tensor_copy(o_sb[:], ps[:])
                    nc.sync.dma_start(
                        out[128 * tc_i:128 * (tc_i + 1),
                            512 * e:512 * (e + 1)], o_sb[:])
    nc.compile()
    return nc


def _masks_np():
    m = np.zeros((4, 128, TB), dtype=np.float32)
    kr = np.arange(128)[:, None]
    qc = np.arange(TB)[None, :]
    for j in range(4):
        m[j] = np.where(kr <= qc - 128 * j, 1.0, 0.0).astype(np.float32)
    return m


def _prep_in_maps(x, w_qkv, w_proj):
    masks = _masks_np()
    in_maps = []
    for c in range(NCORES):
        b, g = c // 4, c % 4
        heads = slice(g * HPC * HD, (g + 1) * HPC * HD)      # 256 rows
        wq = w_qkv[0 * D:1 * D][heads]                        # [256, 1024]
        wk = w_qkv[1 * D:2 * D][heads]
        wv = w_qkv[2 * D:3 * D][heads]
        in_maps.append({
            "xT": np.ascontiguousarray(x[b].T),               # [1024, 2048]
            "wqk": np.ascontiguousarray(
                np.concatenate([wq, wk], axis=0).T),          # [1024, 512]
            "wv": np.ascontiguousarray(wv.T),                 # [1024, 256]
            "wp": np.ascontiguousarray(w_proj[:, heads].T),   # [256, 1024]
            "masks": masks,
            "onesd": np.ones((128, HD), dtype=np.float32),
        })
    return in_maps


def kernel(x, w_qkv, w_proj, _trace=False):
    x = np.asarray(x, dtype=np.float32)
    w_qkv = np.asarray(w_qkv, dtype=np.float32)
    w_proj = np.asarray(w_proj, dtype=np.float32)
    if _trace:
        _ensure_ntff_hook()
    if "nc" not in _cache:
        _cache["nc"] = _build_nc()
    nc = _cache["nc"]
    in_maps = _prep_in_maps(x, w_qkv, w_proj)
    res = run_bass_kernel_spmd(nc, in_maps, list(range(NCORES)),
                               trace=_trace)
    out = np.zeros((B, T, D), dtype=np.float32)
    for c in range(NCORES):
        out[c // 4] += res.results[c]["out"]
    if _trace:
        _cache["last_result"] = res
    return out



# revision 3
# speedup vs baseline: 1.7828x; 1.7828x over previous
"""Causal self-attention Trainium2 kernel (8 NeuronCores).

Sharding: tensor-parallel over heads x data-parallel over batch.
Core c handles batch b = c // 4 and head group g = c % 4 (4 heads of 16).
Each core computes q/k/v projections for its heads, causal attention, and a
partial output projection (its 256 columns of the 1024-wide contraction);
the host sums the 4 partials per batch.

Schedule (v2): single interleaved stream engineered around two facts from
the v1 trace: (a) the ScalarE exp drain paces attention (~1us per kc), and
(b) the PE HAM clock re-throttles to 1.2 GHz whenever the PE micro-idles,
doubling every matmul. Fixes:
  - heads processed in two pair-sweeps per q-block so PSUM fits: 2 PV
    accumulator banks + 2x2-bank score tiles (rotation) + 2 filler banks.
  - scores for a head pair are two K=64 matmuls at base partitions 0/64
    emitted back-to-back -> co-run in opposite PE row groups.
  - one [128, 2, 512] exp per kc (halves ACT instruction overhead).
  - causal narrowing: diag chunk j only computes score/exp/PV columns
    >= min(128j, 256); the mask shrinks to one [128,256] = [zeros|triu]
    multiply on the band.
  - QKV projection of block b+1 and output projection of block b-1 are
    emitted as PE filler inside block b's attention loop so the PE never
    idles (keeps HAM warm) while ACT drains exps.
  - all PSUM drains on VectorE explicitly; den rows on ScalarE; recip +
    normalization on VectorE; partition broadcast on GpSimd.
"""

import collections

import numpy as np

import concourse.bass as bass
from concourse import bacc
import concourse.mybir as mybir
import concourse.tile as tile
from concourse.bass_utils import run_bass_kernel_spmd

B, T, D, H = 2, 2048, 1024, 16
HD = D // H          # 64
HPC = 4              # heads per core
NCORES = 8
EQK = 2 * HPC * HD   # 512 rows of q+k per core
EV = HPC * HD        # 256 rows of v per core
TB = 512             # t/q block
NTB = T // TB        # 4
TC = 128             # t chunk
NTC = T // TC        # 16
DCH = D // 128       # 8 contraction chunks
F32 = mybir.dt.float32
F32R = mybir.dt.float32r
EXP = mybir.ActivationFunctionType.Exp

_cache = {}


def _ensure_ntff_hook():
    """The agent image's antenv lacks axon_hooks; fabricate it so
    run_bass_kernel_spmd(trace=True) can capture NTFF profiles."""
    import sys
    import types
    try:
        import antenv.axon_hooks  # noqa: F401
        return
    except ImportError:
        pass
    try:
        import antenv
        from trn_agent_boot.trn_boot import _ntff_profile_via_ctypes
        hook = {"h": _ntff_profile_via_ctypes("/opt/axon/libaxon_pjrt.so")}
        m = types.ModuleType("antenv.axon_hooks")
        m.get_axon_ntff_profile_hook = lambda: hook["h"]
        m.set_axon_ntff_profile_hook = lambda h: hook.update(h=h)
        sys.modules["antenv.axon_hooks"] = m
        antenv.axon_hooks = m
    except Exception:
        pass


def _build_nc():
    nc = bacc.Bacc("TRN2", target_bir_lowering=False, debug=False,
                  num_devices=NCORES)
    xT = nc.dram_tensor("xT", [D, T], F32R, kind="ExternalInput")
    wqk = nc.dram_tensor("wqk", [D, EQK], F32R, kind="ExternalInput")
    wv = nc.dram_tensor("wv", [D, EV], F32R, kind="ExternalInput")
    wp = nc.dram_tensor("wp", [EV, D], F32R, kind="ExternalInput")
    # [zeros(128x128) | triu(128x128)] causal band mask
    masks = nc.dram_tensor("masks", [128, 256], F32, kind="ExternalInput")
    onesd = nc.dram_tensor("onesd", [128, HD], F32R, kind="ExternalInput")
    out = nc.dram_tensor("out", [T, D], F32, kind="ExternalOutput")

    with tile.TileContext(nc) as tc:
        with (
            nc.allow_low_precision(reason="fp32r matmul inputs; psum stays fp32"),
            tc.tile_pool(name="persist", bufs=1) as persist,
            tc.tile_pool(name="xin", bufs=2) as xin,
            tc.tile_pool(name="work", bufs=4) as work,
            tc.tile_pool(name="probsp", bufs=4) as probsp,
            tc.tile_pool(name="outp", bufs=3) as outp,
            tc.tile_pool(name="ps_s", bufs=2, space="PSUM") as ps_s,
            tc.tile_pool(name="ps_pv", bufs=2, space="PSUM") as ps_pv_pool,
            tc.tile_pool(name="ps_mm", bufs=2, space="PSUM") as ps_mm,
        ):
            # ---- persistent SBUF tensors / input DMAs (critical first) ----
            wqk_sb = persist.tile([128, DCH, EQK], F32R)   # 16KB/part
            nc.sync.dma_start(wqk_sb[:], wqk.rearrange("(c p) e -> p c e", p=128))
            x_tiles = {}

            def dma_x(b):
                t = xin.tile([128, DCH, TB], F32R, tag="x")
                nc.sync.dma_start(
                    t[:], xT[:, b * TB:(b + 1) * TB]
                    .rearrange("(c p) t -> p c t", p=128))
                x_tiles[b] = t

            dma_x(0)
            wv_sb = persist.tile([128, DCH, EV], F32R)     # 8KB/part
            nc.sync.dma_start(wv_sb[:], wv.rearrange("(c p) e -> p c e", p=128))
            mask_sb = persist.tile([128, 256], F32)
            nc.sync.dma_start(mask_sb[:], masks[:, :])
            wp_sb = persist.tile([128, 2, D], F32R)        # 8KB/part
            nc.sync.dma_start(wp_sb[:], wp.rearrange("(c p) e -> p c e", p=128))
            dma_x(1)

            # warm the ACT exp table set while QKV(0) runs
            warm = work.tile([1, 8], F32, tag="warm")
            nc.scalar.activation(warm[:], mask_sb[0:1, 0:8], EXP)

            # qkT[e, t]: 4 chunks of 128 e-rows (q heads 01, q heads 23,
            # k heads 01, k heads 23), each [128, T]
            qkT = [persist.tile([128, T], F32R, tag=f"qkT{i}", name=f"qkT{i}")
                   for i in range(4)]
            # v_sb[t_chunk]: [128, h, 65]; col 64 of each head slot is 1.0
            v_sb = [persist.tile([128, HPC, HD + 1], F32R, tag=f"v{i}",
                                name=f"v{i}")
                    for i in range(NTC)]
            # yT: unnormalized-then-normalized attention output, [hd_all, t]
            yT = [persist.tile([128, T], F32R, tag=f"yT{i}", name=f"yT{i}")
                  for i in range(2)]

            def qT_ap(h):  # [64, T]
                return qkT[h // 2][64 * (h % 2):64 * (h % 2) + 64, :]

            def kT_ap(h):  # [64, T]
                return qkT[2 + h // 2][64 * (h % 2):64 * (h % 2) + 64, :]

            # ---------------- chain emitters (filler units) ----------------
            def emit_qk_chain(b, ec):
                ps = ps_mm.tile([128, TB], F32, tag="mm", name="ps_qk")
                for dc in range(DCH):
                    nc.tensor.matmul(
                        ps[:],
                        (wqk_sb[:, dc, 128 * ec:128 * (ec + 1)]),
                        (x_tiles[b][:, dc, :]),
                        start=(dc == 0), stop=(dc == DCH - 1))
                nc.vector.tensor_copy(qkT[ec][:, b * TB:(b + 1) * TB], ps[:])

            def emit_v_chain(b, t2):
                tc_i = 4 * b + t2
                ps = ps_mm.tile([128, TB], F32, tag="mm", name="ps_v")
                for dc in range(DCH):
                    nc.tensor.matmul(
                        ps[:, 0:EV],
                        (x_tiles[b][:, dc, 128 * t2:128 * (t2 + 1)]),
                        (wv_sb[:, dc, :]),
                        start=(dc == 0), stop=(dc == DCH - 1))
                nc.vector.tensor_copy(
                    v_sb[tc_i][:, :, 0:HD],
                    ps[:, 0:EV].rearrange("p (h f) -> p h f", h=HPC))
                nc.sync.dma_start(v_sb[tc_i][:, :, HD], onesd[:, 0:HPC])

            def emit_proj_chain(tc_i, e):
                ps = ps_mm.tile([128, TB], F32, tag="mm", name="ps_proj")
                for c in range(2):
                    nc.tensor.matmul(
                        ps[:],
                        (yT[c][:, 128 * tc_i:128 * (tc_i + 1)]),
                        (wp_sb[:, c, 512 * e:512 * (e + 1)]),
                        start=(c == 0), stop=(c == 1))
                o_sb = outp.tile([128, TB], F32, tag="o")
                nc.vector.tensor_copy(o_sb[:], ps[:])
                nc.sync.dma_start(
                    out[128 * tc_i:128 * (tc_i + 1),
                        512 * e:512 * (e + 1)], o_sb[:])

            # qkv_fill entries are (block, thunk) and must run before that
            # block's attention; proj_fill can run whenever.
            qkv_fill = collections.deque()
            proj_fill = collections.deque()

            def pop_filler():
                if qkv_fill:
                    qkv_fill.popleft()[1]()
                elif proj_fill:
                    proj_fill.popleft()()

            # ---------------- prologue: QKV(0) ----------------
            for ec in range(4):
                emit_qk_chain(0, ec)
            for t2 in range(4):
                emit_v_chain(0, t2)

            # ---------------- main loop over q-blocks ----------------
            for b in range(NTB):
                nk = 4 * b + 4
                if b + 2 < NTB:
                    dma_x(b + 2)
                if b + 1 < NTB:
                    for ec in range(4):
                        qkv_fill.append(
                            (b + 1,
                             (lambda bb=b + 1, e=ec: emit_qk_chain(bb, e))))
                    for t2 in range(4):
                        qkv_fill.append(
                            (b + 1,
                             (lambda bb=b + 1, t=t2: emit_v_chain(bb, t))))
                if b >= 1:
                    for tci in range(4 * (b - 1), 4 * b):
                        for e in range(2):
                            proj_fill.append(
                                lambda t=tci, ee=e: emit_proj_chain(t, ee))
                # anything queued for block <= b must be emitted before the
                # sweeps that consume its outputs
                while qkv_fill and qkv_fill[0][0] <= b:
                    qkv_fill.popleft()[1]()

                for pair in range(2):
                    pvs = [ps_pv_pool.tile([HD + 1, TB], F32, tag="pv",
                                           name=f"pv{h2}")
                           for h2 in range(2)]
                    for kc in range(nk):
                        j = kc - 4 * b
                        c0 = 0 if j < 0 else min(128 * j, 256)
                        sc = ps_s.tile([128, 2, TB], F32, tag="s", name="sc")
                        for h2 in range(2):
                            h = 2 * pair + h2
                            nc.tensor.matmul(
                                sc[:, h2, c0:TB],
                                (kT_ap(h)[:, 128 * kc:128 * (kc + 1)]),
                                (qT_ap(h)[:, b * TB + c0:(b + 1) * TB]),
                                start=True, stop=True)
                        pr = probsp.tile([128, 2, TB], F32R, tag="p",
                                         name="probs")
                        nc.scalar.activation(
                            pr[:, :, c0:TB], sc[:, :, c0:TB], EXP,
                            scale=1.0 / np.sqrt(HD))
                        if j >= 0:
                            # band mask: j<3 -> triu on [c0,c0+128);
                            # j==3 -> [zeros|triu] on [256,512)
                            w = 256 if j == 3 else 128
                            ms = 0 if j == 3 else 128
                            for h2 in range(2):
                                nc.vector.tensor_mul(
                                    pr[:, h2, c0:c0 + w],
                                    pr[:, h2, c0:c0 + w],
                                    mask_sb[:, ms:ms + w])
                        for h2 in range(2):
                            nc.tensor.matmul(
                                pvs[h2][:, c0:TB],
                                (v_sb[kc][:, 2 * pair + h2, :]),
                                (pr[:, h2, c0:TB]),
                                start=(kc == 0), stop=(kc == nk - 1))
                        if kc % 2 == 1:
                            pop_filler()
                    # sweep drain: unnormalized yT copy first (frees PSUM),
                    # then normalize in SBUF off the bank critical path.
                    for h2 in range(2):
                        h = 2 * pair + h2
                        yslice = yT[h // 2][64 * (h % 2):64 * (h % 2) + 64,
                                            b * TB:(b + 1) * TB]
                        nc.vector.tensor_copy(yslice, pvs[h2][0:HD, :])
                        den = work.tile([1, TB], F32, tag="den",
                                        name=f"den{h2}")
                        nc.scalar.copy(den[:], pvs[h2][HD:HD + 1, :])
                        rec = work.tile([1, TB], F32, tag="rec",
                                        name=f"rec{h2}")
                        nc.vector.reciprocal_approx_fast(rec[:], den[:])
                        bc = work.tile([128, TB], F32, tag="bc")
                        nc.gpsimd.partition_broadcast(bc[:], rec[:])
                        off = 64 * (h % 2)
                        nc.vector.tensor_mul(yslice, yslice,
                                             bc[off:off + 64, :])

            # ---------------- epilogue ----------------
            for tci in range(4 * (NTB - 1), 4 * NTB):
                for e in range(2):
                    proj_fill.append(
                        lambda t=tci, ee=e: emit_proj_chain(t, ee))
            while qkv_fill or proj_fill:
                pop_filler()
    nc.compile()
    return nc


def _mask_np():
    m = np.zeros((128, 256), dtype=np.float32)
    m[:, 128:] = np.triu(np.ones((128, 128), dtype=np.float32))
    return m


def _prep_in_maps(x, w_qkv, w_proj):
    mask = _mask_np()
    in_maps = []
    for c in range(NCORES):
        b, g = c // 4, c % 4
        heads = slice(g * HPC * HD, (g + 1) * HPC * HD)      # 256 rows
        wq = w_qkv[0 * D:1 * D][heads]                        # [256, 1024]
        wk = w_qkv[1 * D:2 * D][heads]
        wv = w_qkv[2 * D:3 * D][heads]
        in_maps.append({
            "xT": np.ascontiguousarray(x[b].T),               # [1024, 2048]
            "wqk": np.ascontiguousarray(
                np.concatenate([wq, wk], axis=0).T),          # [1024, 512]
            "wv": np.ascontiguousarray(wv.T),                 # [1024, 256]
            "wp": np.ascontiguousarray(w_proj[:, heads].T),   # [256, 1024]
            "masks": mask,
            "onesd": np.ones((128, HD), dtype=np.float32),
        })
    return in_maps


def kernel(x, w_qkv, w_proj, _trace=False):
    x = np.asarray(x, dtype=np.float32)
    w_qkv = np.asarray(w_qkv, dtype=np.float32)
    w_proj = np.asarray(w_proj, dtype=np.float32)
    if _trace:
        _ensure_ntff_hook()
    if "nc" not in _cache:
        _cache["nc"] = _build_nc()
    nc = _cache["nc"]
    in_maps = _prep_in_maps(x, w_qkv, w_proj)
    res = run_bass_kernel_spmd(nc, in_maps, list(range(NCORES)),
                               trace=_trace)
    out = np.zeros((B, T, D), dtype=np.float32)
    for c in range(NCORES):
        out[c // 4] += res.results[c]["out"]
    if _trace:
        _cache["last_result"] = res
    return out


# revision 6
# speedup vs baseline: 1.7838x; 1.0006x over previous
"""Causal self-attention Trainium2 kernel (8 NeuronCores).

Sharding: tensor-parallel over heads x data-parallel over batch.
Core c handles batch b = c // 4 and head group g = c % 4 (4 heads of 16).
Each core computes q/k/v projections for its heads, causal attention, and a
partial output projection (its 256 columns of the 1024-wide contraction);
the host sums the 4 partials per batch.

Schedule (v2): single interleaved stream engineered around two facts from
the v1 trace: (a) the ScalarE exp drain paces attention (~1us per kc), and
(b) the PE HAM clock re-throttles to 1.2 GHz whenever the PE micro-idles,
doubling every matmul. Fixes:
  - heads processed in two pair-sweeps per q-block so PSUM fits: 2 PV
    accumulator banks + 2x2-bank score tiles (rotation) + 2 filler banks.
  - scores for a head pair are two K=64 matmuls at base partitions 0/64
    emitted back-to-back -> co-run in opposite PE row groups.
  - one [128, 2, 512] exp per kc (halves ACT instruction overhead).
  - causal narrowing: diag chunk j only computes score/exp/PV columns
    >= min(128j, 256); the mask shrinks to one [128,256] = [zeros|triu]
    multiply on the band.
  - QKV projection of block b+1 and output projection of block b-1 are
    emitted as PE filler inside block b's attention loop so the PE never
    idles (keeps HAM warm) while ACT drains exps.
  - all PSUM drains on VectorE explicitly; den rows on ScalarE; recip +
    normalization on VectorE; partition broadcast on GpSimd.
"""

import collections

import numpy as np

import concourse.bass as bass
from concourse import bacc
import concourse.mybir as mybir
import concourse.tile as tile
from concourse.bass_utils import run_bass_kernel_spmd

B, T, D, H = 2, 2048, 1024, 16
HD = D // H          # 64
HPC = 4              # heads per core
NCORES = 8
EQK = 2 * HPC * HD   # 512 rows of q+k per core
EV = HPC * HD        # 256 rows of v per core
TB = 512             # t/q block
NTB = T // TB        # 4
TC = 128             # t chunk
NTC = T // TC        # 16
DCH = D // 128       # 8 contraction chunks
F32 = mybir.dt.float32
F32R = mybir.dt.float32r
EXP = mybir.ActivationFunctionType.Exp

_cache = {}


def _ensure_ntff_hook():
    """The agent image's antenv lacks axon_hooks; fabricate it so
    run_bass_kernel_spmd(trace=True) can capture NTFF profiles."""
    import sys
    import types
    try:
        import antenv.axon_hooks  # noqa: F401
        return
    except ImportError:
        pass
    try:
        import antenv
        from trn_agent_boot.trn_boot import _ntff_profile_via_ctypes
        hook = {"h": _ntff_profile_via_ctypes("/opt/axon/libaxon_pjrt.so")}
        m = types.ModuleType("antenv.axon_hooks")
        m.get_axon_ntff_profile_hook = lambda: hook["h"]
        m.set_axon_ntff_profile_hook = lambda h: hook.update(h=h)
        sys.modules["antenv.axon_hooks"] = m
        antenv.axon_hooks = m
    except Exception:
        pass


def _build_nc():
    nc = bacc.Bacc("TRN2", target_bir_lowering=False, debug=False,
                  num_devices=NCORES)
    xT = nc.dram_tensor("xT", [D, T], F32R, kind="ExternalInput")
    wqk = nc.dram_tensor("wqk", [D, EQK], F32R, kind="ExternalInput")
    wv = nc.dram_tensor("wv", [D, EV], F32R, kind="ExternalInput")
    wp = nc.dram_tensor("wp", [EV, D], F32R, kind="ExternalInput")
    # [zeros(128x128) | triu(128x128)] causal band mask
    masks = nc.dram_tensor("masks", [128, 256], F32, kind="ExternalInput")
    onesd = nc.dram_tensor("onesd", [128, HD], F32R, kind="ExternalInput")
    out = nc.dram_tensor("out", [T, D], F32, kind="ExternalOutput")

    with tile.TileContext(nc) as tc:
        with (
            nc.allow_low_precision(reason="fp32r matmul inputs; psum stays fp32"),
            tc.tile_pool(name="persist", bufs=1) as persist,
            tc.tile_pool(name="xin", bufs=2) as xin,
            tc.tile_pool(name="work", bufs=4) as work,
            tc.tile_pool(name="probsp", bufs=4) as probsp,
            tc.tile_pool(name="outp", bufs=3) as outp,
            tc.tile_pool(name="ps_s", bufs=2, space="PSUM") as ps_s,
            tc.tile_pool(name="ps_pv", bufs=2, space="PSUM") as ps_pv_pool,
            tc.tile_pool(name="ps_mm", bufs=2, space="PSUM") as ps_mm,
        ):
            # ---- persistent SBUF tensors / input DMAs ----
            # Big DMAs are split per 128-row chunk so the first qk chain can
            # start after ~256KB arrives instead of waiting for 4MB.
            wqk_sb = persist.tile([128, DCH, EQK], F32R)   # 16KB/part
            x_tiles = {}

            def dma_x(b, split=False):
                t = xin.tile([128, DCH, TB], F32R, tag="x")
                if split:
                    for dc in range(DCH):
                        nc.sync.dma_start(
                            t[:, dc, :],
                            xT[128 * dc:128 * (dc + 1),
                               b * TB:(b + 1) * TB])
                else:
                    nc.sync.dma_start(
                        t[:], xT[:, b * TB:(b + 1) * TB]
                        .rearrange("(c p) t -> p c t", p=128))
                x_tiles[b] = t

            x_tiles[0] = xin.tile([128, DCH, TB], F32R, tag="x", name="x0")
            for dc in range(DCH):
                nc.sync.dma_start(
                    wqk_sb[:, dc, :], wqk[128 * dc:128 * (dc + 1), :])
                nc.sync.dma_start(
                    x_tiles[0][:, dc, :], xT[128 * dc:128 * (dc + 1), 0:TB])
            wv_sb = persist.tile([128, DCH, EV], F32R)     # 8KB/part
            for dc in range(DCH):
                nc.sync.dma_start(
                    wv_sb[:, dc, :], wv[128 * dc:128 * (dc + 1), :])
            mask_sb = persist.tile([128, 256], F32)
            nc.sync.dma_start(mask_sb[:], masks[:, :])
            wp_sb = persist.tile([128, 2, D], F32R)        # 8KB/part
            nc.sync.dma_start(wp_sb[:], wp.rearrange("(c p) e -> p c e", p=128))
            dma_x(1)

            # warm the ACT exp table set while QKV(0) runs
            warm = work.tile([1, 8], F32, tag="warm")
            nc.scalar.activation(warm[:], mask_sb[0:1, 0:8], EXP)

            # qkT[e, t]: 4 chunks of 128 e-rows (q heads 01, q heads 23,
            # k heads 01, k heads 23), each [128, T]
            qkT = [persist.tile([128, T], F32R, tag=f"qkT{i}", name=f"qkT{i}")
                   for i in range(4)]
            # v_sb[t_chunk]: [128, h, 65]; col 64 of each head slot is 1.0
            v_sb = [persist.tile([128, HPC, HD + 1], F32R, tag=f"v{i}",
                                name=f"v{i}")
                    for i in range(NTC)]
            # yT: unnormalized-then-normalized attention output, [hd_all, t]
            yT = [persist.tile([128, T], F32R, tag=f"yT{i}", name=f"yT{i}")
                  for i in range(2)]

            def qT_ap(h):  # [64, T]
                return qkT[h // 2][64 * (h % 2):64 * (h % 2) + 64, :]

            def kT_ap(h):  # [64, T]
                return qkT[2 + h // 2][64 * (h % 2):64 * (h % 2) + 64, :]

            # ---------------- chain emitters (filler units) ----------------
            def emit_qk_chain(b, ec):
                ps = ps_mm.tile([128, TB], F32, tag="mm", name="ps_qk")
                for dc in range(DCH):
                    nc.tensor.matmul(
                        ps[:],
                        (wqk_sb[:, dc, 128 * ec:128 * (ec + 1)]),
                        (x_tiles[b][:, dc, :]),
                        start=(dc == 0), stop=(dc == DCH - 1))
                nc.vector.tensor_copy(qkT[ec][:, b * TB:(b + 1) * TB], ps[:])

            def emit_v_chain(b, t2):
                tc_i = 4 * b + t2
                ps = ps_mm.tile([128, TB], F32, tag="mm", name="ps_v")
                for dc in range(DCH):
                    nc.tensor.matmul(
                        ps[:, 0:EV],
                        (x_tiles[b][:, dc, 128 * t2:128 * (t2 + 1)]),
                        (wv_sb[:, dc, :]),
                        start=(dc == 0), stop=(dc == DCH - 1))
                nc.vector.tensor_copy(
                    v_sb[tc_i][:, :, 0:HD],
                    ps[:, 0:EV].rearrange("p (h f) -> p h f", h=HPC))
                nc.sync.dma_start(v_sb[tc_i][:, :, HD], onesd[:, 0:HPC])

            def emit_proj_chain(tc_i, e):
                ps = ps_mm.tile([128, TB], F32, tag="mm", name="ps_proj")
                for c in range(2):
                    nc.tensor.matmul(
                        ps[:],
                        (yT[c][:, 128 * tc_i:128 * (tc_i + 1)]),
                        (wp_sb[:, c, 512 * e:512 * (e + 1)]),
                        start=(c == 0), stop=(c == 1))
                o_sb = outp.tile([128, TB], F32, tag="o")
                nc.vector.tensor_copy(o_sb[:], ps[:])
                nc.sync.dma_start(
                    out[128 * tc_i:128 * (tc_i + 1),
                        512 * e:512 * (e + 1)], o_sb[:])

            # qkv_fill entries are (block, thunk) and must run before that
            # block's attention; proj_fill can run whenever.
            qkv_fill = collections.deque()
            proj_fill = collections.deque()

            def pop_filler():
                if qkv_fill:
                    qkv_fill.popleft()[1]()
                elif proj_fill:
                    proj_fill.popleft()()

            # ---------------- prologue: QKV(0) ----------------
            for ec in range(4):
                emit_qk_chain(0, ec)
            for t2 in range(4):
                emit_v_chain(0, t2)

            # ---------------- main loop over q-blocks ----------------
            for b in range(NTB):
                nk = 4 * b + 4
                if b + 2 < NTB:
                    dma_x(b + 2)
                if b + 1 < NTB:
                    for ec in range(4):
                        qkv_fill.append(
                            (b + 1,
                             (lambda bb=b + 1, e=ec: emit_qk_chain(bb, e))))
                    for t2 in range(4):
                        qkv_fill.append(
                            (b + 1,
                             (lambda bb=b + 1, t=t2: emit_v_chain(bb, t))))
                if b >= 1:
                    for tci in range(4 * (b - 1), 4 * b):
                        for e in range(2):
                            proj_fill.append(
                                lambda t=tci, ee=e: emit_proj_chain(t, ee))
                # anything queued for block <= b must be emitted before the
                # sweeps that consume its outputs
                while qkv_fill and qkv_fill[0][0] <= b:
                    qkv_fill.popleft()[1]()

                for pair in range(2):
                    pvs = [ps_pv_pool.tile([HD + 1, TB], F32, tag="pv",
                                           name=f"pv{h2}")
                           for h2 in range(2)]
                    for kc in range(nk):
                        j = kc - 4 * b
                        c0 = 0 if j < 0 else min(128 * j, 256)
                        sc = ps_s.tile([128, 2, TB], F32, tag="s", name="sc")
                        for h2 in range(2):
                            h = 2 * pair + h2
                            nc.tensor.matmul(
                                sc[:, h2, c0:TB],
                                (kT_ap(h)[:, 128 * kc:128 * (kc + 1)]),
                                (qT_ap(h)[:, b * TB + c0:(b + 1) * TB]),
                                start=True, stop=True)
                        pr = probsp.tile([128, 2, TB], F32R, tag="p",
                                         name="probs")
                        nc.scalar.activation(
                            pr[:, :, c0:TB], sc[:, :, c0:TB], EXP,
                            scale=1.0 / np.sqrt(HD))
                        if j >= 0:
                            # band mask: j<3 -> triu on [c0,c0+128);
                            # j==3 -> [zeros|triu] on [256,512)
                            w = 256 if j == 3 else 128
                            ms = 0 if j == 3 else 128
                            for h2 in range(2):
                                nc.vector.tensor_mul(
                                    pr[:, h2, c0:c0 + w],
                                    pr[:, h2, c0:c0 + w],
                                    mask_sb[:, ms:ms + w])
                        for h2 in range(2):
                            nc.tensor.matmul(
                                pvs[h2][:, c0:TB],
                                (v_sb[kc][:, 2 * pair + h2, :]),
                                (pr[:, h2, c0:TB]),
                                start=(kc == 0), stop=(kc == nk - 1))
                        if kc % 3 == 2:
                            pop_filler()
                    # sweep drain: unnormalized yT copy first (frees PSUM),
                    # then normalize in SBUF off the bank critical path.
                    for h2 in range(2):
                        h = 2 * pair + h2
                        yslice = yT[h // 2][64 * (h % 2):64 * (h % 2) + 64,
                                            b * TB:(b + 1) * TB]
                        nc.vector.tensor_copy(yslice, pvs[h2][0:HD, :])
                        den = work.tile([1, TB], F32, tag="den",
                                        name=f"den{h2}")
                        nc.scalar.copy(den[:], pvs[h2][HD:HD + 1, :])
                        rec = work.tile([1, TB], F32, tag="rec",
                                        name=f"rec{h2}")
                        nc.vector.reciprocal_approx_fast(rec[:], den[:])
                        bc = work.tile([128, TB], F32, tag="bc")
                        nc.gpsimd.partition_broadcast(bc[:], rec[:])
                        off = 64 * (h % 2)
                        nc.vector.tensor_mul(yslice, yslice,
                                             bc[off:off + 64, :])

            # ---------------- epilogue ----------------
            for tci in range(4 * (NTB - 1), 4 * NTB):
                for e in range(2):
                    proj_fill.append(
                        lambda t=tci, ee=e: emit_proj_chain(t, ee))
            while qkv_fill or proj_fill:
                pop_filler()
    nc.compile()
    return nc


def _mask_np():
    m = np.zeros((128, 256), dtype=np.float32)
    m[:, 128:] = np.triu(np.ones((128, 128), dtype=np.float32))
    return m


def _prep_in_maps(x, w_qkv, w_proj):
    mask = _mask_np()
    in_maps = []
    for c in range(NCORES):
        b, g = c // 4, c % 4
        heads = slice(g * HPC * HD, (g + 1) * HPC * HD)      # 256 rows
        wq = w_qkv[0 * D:1 * D][heads]                        # [256, 1024]
        wk = w_qkv[1 * D:2 * D][heads]
        wv = w_qkv[2 * D:3 * D][heads]
        in_maps.append({
            "xT": np.ascontiguousarray(x[b].T),               # [1024, 2048]
            "wqk": np.ascontiguousarray(
                np.concatenate([wq, wk], axis=0).T),          # [1024, 512]
            "wv": np.ascontiguousarray(wv.T),                 # [1024, 256]
            "wp": np.ascontiguousarray(w_proj[:, heads].T),   # [256, 1024]
            "masks": mask,
            "onesd": np.ones((128, HD), dtype=np.float32),
        })
    return in_maps


def kernel(x, w_qkv, w_proj, _trace=False):
    x = np.asarray(x, dtype=np.float32)
    w_qkv = np.asarray(w_qkv, dtype=np.float32)
    w_proj = np.asarray(w_proj, dtype=np.float32)
    if _trace:
        _ensure_ntff_hook()
    if "nc" not in _cache:
        _cache["nc"] = _build_nc()
    nc = _cache["nc"]
    in_maps = _prep_in_maps(x, w_qkv, w_proj)
    res = run_bass_kernel_spmd(nc, in_maps, list(range(NCORES)),
                               trace=_trace)
    out = np.zeros((B, T, D), dtype=np.float32)
    for c in range(NCORES):
        out[c // 4] += res.results[c]["out"]
    if _trace:
        _cache["last_result"] = res
    return out


# revision 10
# speedup vs baseline: 1.7997x; 1.0089x over previous
"""Causal self-attention Trainium2 kernel (8 NeuronCores).

Sharding: tensor-parallel over heads x data-parallel over batch.
Core c handles batch b = c // 4 and head group g = c % 4 (4 heads of 16).
Each core computes q/k/v projections for its heads, causal attention, and a
partial output projection (its 256 columns of the 1024-wide contraction);
the host sums the 4 partials per batch.

Schedule (v2): single interleaved stream engineered around two facts from
the v1 trace: (a) the ScalarE exp drain paces attention (~1us per kc), and
(b) the PE HAM clock re-throttles to 1.2 GHz whenever the PE micro-idles,
doubling every matmul. Fixes:
  - heads processed in two pair-sweeps per q-block so PSUM fits: 2 PV
    accumulator banks + 2x2-bank score tiles (rotation) + 2 filler banks.
  - scores for a head pair are two K=64 matmuls at base partitions 0/64
    emitted back-to-back -> co-run in opposite PE row groups.
  - one [128, 2, 512] exp per kc (halves ACT instruction overhead).
  - causal narrowing: diag chunk j only computes score/exp/PV columns
    >= min(128j, 256); the mask shrinks to one [128,256] = [zeros|triu]
    multiply on the band.
  - QKV projection of block b+1 and output projection of block b-1 are
    emitted as PE filler inside block b's attention loop so the PE never
    idles (keeps HAM warm) while ACT drains exps.
  - all PSUM drains on VectorE explicitly; den rows on ScalarE; recip +
    normalization on VectorE; partition broadcast on GpSimd.
"""

import collections

import numpy as np

import concourse.bass as bass
from concourse import bacc
import concourse.mybir as mybir
import concourse.tile as tile
from concourse.bass_utils import run_bass_kernel_spmd

B, T, D, H = 2, 2048, 1024, 16
HD = D // H          # 64
HPC = 4              # heads per core
NCORES = 8
EQK = 2 * HPC * HD   # 512 rows of q+k per core
EV = HPC * HD        # 256 rows of v per core
TB = 512             # t/q block
NTB = T // TB        # 4
TC = 128             # t chunk
NTC = T // TC        # 16
DCH = D // 128       # 8 contraction chunks
F32 = mybir.dt.float32
F32R = mybir.dt.float32r
EXP = mybir.ActivationFunctionType.Exp

_cache = {}


def _ensure_ntff_hook():
    """The agent image's antenv lacks axon_hooks; fabricate it so
    run_bass_kernel_spmd(trace=True) can capture NTFF profiles."""
    import sys
    import types
    try:
        import antenv.axon_hooks  # noqa: F401
        return
    except ImportError:
        pass
    try:
        import antenv
        from trn_agent_boot.trn_boot import _ntff_profile_via_ctypes
        hook = {"h": _ntff_profile_via_ctypes("/opt/axon/libaxon_pjrt.so")}
        m = types.ModuleType("antenv.axon_hooks")
        m.get_axon_ntff_profile_hook = lambda: hook["h"]
        m.set_axon_ntff_profile_hook = lambda h: hook.update(h=h)
        sys.modules["antenv.axon_hooks"] = m
        antenv.axon_hooks = m
    except Exception:
        pass


def _build_nc():
    nc = bacc.Bacc("TRN2", target_bir_lowering=False, debug=False,
                  num_devices=NCORES)
    xT = nc.dram_tensor("xT", [D, T], F32R, kind="ExternalInput")
    wqk = nc.dram_tensor("wqk", [D, EQK], F32R, kind="ExternalInput")
    wv = nc.dram_tensor("wv", [D, EV], F32R, kind="ExternalInput")
    wp = nc.dram_tensor("wp", [EV, D], F32R, kind="ExternalInput")
    # [zeros(128x128) | triu(128x128)] causal band mask
    masks = nc.dram_tensor("masks", [128, 256], F32, kind="ExternalInput")
    onesd = nc.dram_tensor("onesd", [128, HD], F32R, kind="ExternalInput")
    out = nc.dram_tensor("out", [T, D], F32, kind="ExternalOutput")

    with tile.TileContext(nc) as tc:
        with (
            nc.allow_low_precision(reason="fp32r matmul inputs; psum stays fp32"),
            tc.tile_pool(name="persist", bufs=1) as persist,
            tc.tile_pool(name="xin", bufs=2) as xin,
            tc.tile_pool(name="work", bufs=4) as work,
            tc.tile_pool(name="probsp", bufs=4) as probsp,
            tc.tile_pool(name="outp", bufs=3) as outp,
            tc.tile_pool(name="ps_s", bufs=2, space="PSUM") as ps_s,
            tc.tile_pool(name="ps_pv", bufs=2, space="PSUM") as ps_pv_pool,
            tc.tile_pool(name="ps_mm", bufs=2, space="PSUM") as ps_mm,
        ):
            # ---- persistent SBUF tensors / input DMAs ----
            # Big DMAs are split per 128-row chunk so the first qk chain can
            # start after ~256KB arrives instead of waiting for 4MB.
            wqk_sb = persist.tile([128, DCH, EQK], F32R)   # 16KB/part
            x_tiles = {}

            def dma_x(b, split=False):
                t = xin.tile([128, DCH, TB], F32R, tag="x")
                if split:
                    for dc in range(DCH):
                        nc.sync.dma_start(
                            t[:, dc, :],
                            xT[128 * dc:128 * (dc + 1),
                               b * TB:(b + 1) * TB])
                else:
                    nc.sync.dma_start(
                        t[:], xT[:, b * TB:(b + 1) * TB]
                        .rearrange("(c p) t -> p c t", p=128))
                x_tiles[b] = t

            x_tiles[0] = xin.tile([128, DCH, TB], F32R, tag="x", name="x0")
            for dc in range(DCH):
                nc.sync.dma_start(
                    wqk_sb[:, dc, :], wqk[128 * dc:128 * (dc + 1), :])
                nc.sync.dma_start(
                    x_tiles[0][:, dc, :], xT[128 * dc:128 * (dc + 1), 0:TB])
            wv_sb = persist.tile([128, DCH, EV], F32R)     # 8KB/part
            for dc in range(DCH):
                nc.sync.dma_start(
                    wv_sb[:, dc, :], wv[128 * dc:128 * (dc + 1), :])
            mask_sb = persist.tile([128, 256], F32)
            nc.sync.dma_start(mask_sb[:], masks[:, :])
            wp_sb = persist.tile([128, 2, D], F32R)        # 8KB/part
            nc.sync.dma_start(wp_sb[:], wp.rearrange("(c p) e -> p c e", p=128))
            dma_x(1)

            # warm the ACT exp table set while QKV(0) runs
            warm = work.tile([1, 8], F32, tag="warm")
            nc.scalar.activation(warm[:], mask_sb[0:1, 0:8], EXP)

            # warm the PE HAM clock gate while the first DMAs trickle in:
            # ~24 dummy matmuls on (uninitialized) SBUF keep the PE busy
            # through the 3.4us activity window so real matmuls start at
            # 2.4 GHz instead of 1.2.
            dummy_in = persist.tile([128, TB], F32, tag="dummy_in")
            nc.gpsimd.memset(dummy_in[:], 0.0)
            for i in range(24):
                ps_d = ps_s.tile([128, 2, TB], F32, tag="s", name="ps_d")
                nc.tensor.matmul(ps_d[:, 0, 0:128], dummy_in[:, 0:128],
                                 dummy_in[:, 0:128], start=True, stop=True)

            # qkT[e, t]: 4 chunks of 128 e-rows (q heads 01, q heads 23,
            # k heads 01, k heads 23), each [128, T]
            qkT = [persist.tile([128, T], F32R, tag=f"qkT{i}", name=f"qkT{i}")
                   for i in range(4)]
            # v_sb[t_chunk]: [128, h, 65]; col 64 of each head slot is 1.0
            v_sb = [persist.tile([128, HPC, HD + 1], F32R, tag=f"v{i}",
                                name=f"v{i}")
                    for i in range(NTC)]
            # yT: unnormalized-then-normalized attention output, [hd_all, t]
            yT = [persist.tile([128, T], F32R, tag=f"yT{i}", name=f"yT{i}")
                  for i in range(2)]

            def qT_ap(h):  # [64, T]
                return qkT[h // 2][64 * (h % 2):64 * (h % 2) + 64, :]

            def kT_ap(h):  # [64, T]
                return qkT[2 + h // 2][64 * (h % 2):64 * (h % 2) + 64, :]

            # ---------------- chain emitters (filler units) ----------------
            def emit_qk_chain(b, ec):
                ps = ps_mm.tile([128, TB], F32, tag="mm", name="ps_qk")
                for dc in range(DCH):
                    nc.tensor.matmul(
                        ps[:],
                        (wqk_sb[:, dc, 128 * ec:128 * (ec + 1)]),
                        (x_tiles[b][:, dc, :]),
                        start=(dc == 0), stop=(dc == DCH - 1))
                nc.vector.tensor_copy(qkT[ec][:, b * TB:(b + 1) * TB], ps[:])

            def emit_v_chain(b, t2):
                tc_i = 4 * b + t2
                ps = ps_mm.tile([128, TB], F32, tag="mm", name="ps_v")
                for dc in range(DCH):
                    nc.tensor.matmul(
                        ps[:, 0:EV],
                        (x_tiles[b][:, dc, 128 * t2:128 * (t2 + 1)]),
                        (wv_sb[:, dc, :]),
                        start=(dc == 0), stop=(dc == DCH - 1))
                nc.vector.tensor_copy(
                    v_sb[tc_i][:, :, 0:HD],
                    ps[:, 0:EV].rearrange("p (h f) -> p h f", h=HPC))
                nc.sync.dma_start(v_sb[tc_i][:, :, HD], onesd[:, 0:HPC])

            def emit_proj_chain(tc_i, e):
                ps = ps_mm.tile([128, TB], F32, tag="mm", name="ps_proj")
                for c in range(2):
                    nc.tensor.matmul(
                        ps[:],
                        (yT[c][:, 128 * tc_i:128 * (tc_i + 1)]),
                        (wp_sb[:, c, 512 * e:512 * (e + 1)]),
                        start=(c == 0), stop=(c == 1))
                o_sb = outp.tile([128, TB], F32, tag="o")
                nc.vector.tensor_copy(o_sb[:], ps[:])
                nc.sync.dma_start(
                    out[128 * tc_i:128 * (tc_i + 1),
                        512 * e:512 * (e + 1)], o_sb[:])

            # qkv_fill entries are (block, thunk) and must run before that
            # block's attention; proj_fill can run whenever.
            qkv_fill = collections.deque()
            proj_fill = collections.deque()

            def pop_filler():
                if qkv_fill:
                    qkv_fill.popleft()[1]()
                elif proj_fill:
                    proj_fill.popleft()()

            # ---------------- prologue: QKV(0) ----------------
            for ec in range(4):
                emit_qk_chain(0, ec)
            for t2 in range(4):
                emit_v_chain(0, t2)

            # ---------------- main loop over q-blocks ----------------
            for b in range(NTB):
                nk = 4 * b + 4
                if b + 2 < NTB:
                    dma_x(b + 2)
                if b + 1 < NTB:
                    for ec in range(4):
                        qkv_fill.append(
                            (b + 1,
                             (lambda bb=b + 1, e=ec: emit_qk_chain(bb, e))))
                    for t2 in range(4):
                        qkv_fill.append(
                            (b + 1,
                             (lambda bb=b + 1, t=t2: emit_v_chain(bb, t))))
                if b >= 1:
                    for tci in range(4 * (b - 1), 4 * b):
                        for e in range(2):
                            proj_fill.append(
                                lambda t=tci, ee=e: emit_proj_chain(t, ee))
                # anything queued for block <= b must be emitted before the
                # sweeps that consume its outputs
                while qkv_fill and qkv_fill[0][0] <= b:
                    qkv_fill.popleft()[1]()

                for pair in range(2):
                    pvs = [ps_pv_pool.tile([HD + 1, TB], F32, tag="pv",
                                           name=f"pv{h2}")
                           for h2 in range(2)]
                    for kc in range(nk):
                        j = kc - 4 * b
                        c0 = 0 if j < 0 else min(128 * j, 256)
                        sc = ps_s.tile([128, 2, TB], F32, tag="s", name="sc")
                        for h2 in range(2):
                            h = 2 * pair + h2
                            nc.tensor.matmul(
                                sc[:, h2, c0:TB],
                                (kT_ap(h)[:, 128 * kc:128 * (kc + 1)]),
                                (qT_ap(h)[:, b * TB + c0:(b + 1) * TB]),
                                start=True, stop=True)
                        pr = probsp.tile([128, 2, TB], F32R, tag="p",
                                         name="probs")
                        nc.scalar.activation(
                            pr[:, :, c0:TB], sc[:, :, c0:TB], EXP,
                            scale=1.0 / np.sqrt(HD))
                        if j >= 0:
                            # band mask: j<3 -> triu on [c0,c0+128);
                            # j==3 -> [zeros|triu] on [256,512)
                            w = 256 if j == 3 else 128
                            ms = 0 if j == 3 else 128
                            for h2 in range(2):
                                nc.vector.tensor_mul(
                                    pr[:, h2, c0:c0 + w],
                                    pr[:, h2, c0:c0 + w],
                                    mask_sb[:, ms:ms + w])
                        for h2 in range(2):
                            nc.tensor.matmul(
                                pvs[h2][:, c0:TB],
                                (v_sb[kc][:, 2 * pair + h2, :]),
                                (pr[:, h2, c0:TB]),
                                start=(kc == 0), stop=(kc == nk - 1))
                        pace = 2 if b == NTB - 1 else 3
                        if kc % pace == pace - 1:
                            pop_filler()
                    # sweep drain: unnormalized yT copy first (frees PSUM),
                    # then normalize in SBUF off the bank critical path.
                    for h2 in range(2):
                        h = 2 * pair + h2
                        yslice = yT[h // 2][64 * (h % 2):64 * (h % 2) + 64,
                                            b * TB:(b + 1) * TB]
                        nc.vector.tensor_copy(yslice, pvs[h2][0:HD, :])
                        den = work.tile([1, TB], F32, tag="den",
                                        name=f"den{h2}")
                        nc.scalar.copy(den[:], pvs[h2][HD:HD + 1, :])
                        rec = work.tile([1, TB], F32, tag="rec",
                                        name=f"rec{h2}")
                        nc.vector.reciprocal_approx_fast(rec[:], den[:])
                        bc = work.tile([128, TB], F32, tag="bc")
                        nc.gpsimd.partition_broadcast(bc[:], rec[:])
                        off = 64 * (h % 2)
                        nc.vector.tensor_mul(yslice, yslice,
                                             bc[off:off + 64, :])

            # ---------------- epilogue ----------------
            for tci in range(4 * (NTB - 1), 4 * NTB):
                for e in range(2):
                    proj_fill.append(
                        lambda t=tci, ee=e: emit_proj_chain(t, ee))
            while qkv_fill or proj_fill:
                pop_filler()
    nc.compile()
    return nc


def _mask_np():
    m = np.zeros((128, 256), dtype=np.float32)
    m[:, 128:] = np.triu(np.ones((128, 128), dtype=np.float32))
    return m


def _prep_in_maps(x, w_qkv, w_proj):
    mask = _mask_np()
    in_maps = []
    for c in range(NCORES):
        b, g = c // 4, c % 4
        heads = slice(g * HPC * HD, (g + 1) * HPC * HD)      # 256 rows
        wq = w_qkv[0 * D:1 * D][heads]                        # [256, 1024]
        wk = w_qkv[1 * D:2 * D][heads]
        wv = w_qkv[2 * D:3 * D][heads]
        in_maps.append({
            "xT": np.ascontiguousarray(x[b].T),               # [1024, 2048]
            "wqk": np.ascontiguousarray(
                np.concatenate([wq, wk], axis=0).T),          # [1024, 512]
            "wv": np.ascontiguousarray(wv.T),                 # [1024, 256]
            "wp": np.ascontiguousarray(w_proj[:, heads].T),   # [256, 1024]
            "masks": mask,
            "onesd": np.ones((128, HD), dtype=np.float32),
        })
    return in_maps


def kernel(x, w_qkv, w_proj, _trace=False):
    x = np.asarray(x, dtype=np.float32)
    w_qkv = np.asarray(w_qkv, dtype=np.float32)
    w_proj = np.asarray(w_proj, dtype=np.float32)
    if _trace:
        _ensure_ntff_hook()
    if "nc" not in _cache:
        _cache["nc"] = _build_nc()
    nc = _cache["nc"]
    in_maps = _prep_in_maps(x, w_qkv, w_proj)
    res = run_bass_kernel_spmd(nc, in_maps, list(range(NCORES)),
                               trace=_trace)
    out = np.zeros((B, T, D), dtype=np.float32)
    for c in range(NCORES):
        out[c // 4] += res.results[c]["out"]
    if _trace:
        _cache["last_result"] = res
    return out
